# revision 46
# baseline (speedup 1.0000x reference)
"""MHA (projections + masked softmax attention) on 8 NeuronCores.

Data-parallel over batch (B=8 -> 1 batch element per core, no collectives).
bf16 matmul operands (fp32 PSUM accumulation + fp32 softmax normalization).

Per core, transposed layout:
  QT = Wq^T @ x_q^T   [D, Sq]   (lhsT = Wq natural, rhs = x_q^T from host)
  KT = Wk^T @ x_k^T   [D, Sk]
  V  = x_v  @ Wv      [Sk, D]   (lhsT = x_v^T chunk, rhs = Wv natural)

Engines execute their queues in program order, so overlap is created by
EMISSION-ORDER software pipelining (V projection first, then 8
iterations, each weaving Q/K projection chunk `oc` with the attention of
head pair `oc-1`): the ~10us/pair of ACT exp work hides entirely under
~16us/pair of PE work instead of serializing after the projections.

Head pair (2h, 2h+1) shares one 128-partition qt/kt chunk: scores
matmuls have K=64 contraction at PE row offsets 0/64 (tile_position row
tiling -> concurrent on the array), writing BOTH heads into one 2-bank
PSUM piece tile so a single exp (and a single broadcast Pool mask mul)
drains the pair -- ACT/Pool instruction counts halve vs per-head ops.
Scores flow through a 2-deep PSUM piece ring; e-tiles are per-kc pools
sized [128, 2*(S-c0)] with bufs=2 (two pairs in flight) so exp never
waits on the previous pair's AV reads. O^T[d,q] & Z[q] accumulate in ONE
matmul per head (lhsT = [V_h | ones]); finalize = PE transpose + one
reciprocal + one broadcast normalize mul + strided output DMA.

Host: transposes, sort queries by valid_len (column-suffix skipping of
fully-masked key chunks + narrow predication ranges), bf16 0/1 mask
(only the ragged [c0:cv) range is loaded), exact fixup of valid_len==0
rows (reference -> uniform softmax -> mean(value)@Wv).
"""

import os
import sys

if "/opt/trn_rl_repo" not in sys.path:
    sys.path.insert(0, "/opt/trn_rl_repo")

import numpy as np

ABLATE = set(os.environ.get("ABLATE", "").split(","))

B, S, D, H = 8, 1024, 1024, 16
DH = D // H  # 64
P = 128
KC = S // P  # 8 key chunks
DC = D // P  # 8 hidden chunks
N_CORES = 8
NEG = -480.0  # exp(0.125 * -480) = exp(-60) ~= 8.8e-27
TRS = 128  # per-chunk column stride in the transpose staging tile


def _build_nc(col_start, pred_end, reps=1):
    """col_start[kc]: first sorted-q column (mult of 128, 0..1024) needing
    key-chunk kc (1024 = chunk skipped). pred_end[kc]: end (exclusive, mult
    of 32) of the mask-predication range. Unions over cores. col_start[0]
    must be 0."""
    from contextlib import ExitStack

    import concourse.mybir as mybir
    import concourse.tile as tile
    from concourse import bacc
    from concourse.masks import make_identity

    fp32 = mybir.dt.float32
    bf16 = mybir.dt.bfloat16
    u8 = mybir.dt.uint8
    AF = mybir.ActivationFunctionType

    nc = bacc.Bacc(
        "TRN2",
        target_bir_lowering=False,
        debug=False,
        enable_asserts=False,
        num_devices=N_CORES,
    )

    xqT = nc.dram_tensor("xqT", (D, S), bf16, kind="ExternalInput").ap()
    xkT = nc.dram_tensor("xkT", (D, S), bf16, kind="ExternalInput").ap()
    xvT = nc.dram_tensor("xvT", (D, S), bf16, kind="ExternalInput").ap()
    wq = nc.dram_tensor("wq", (D, D), bf16, kind="ExternalInput").ap()
    wk = nc.dram_tensor("wk", (D, D), bf16, kind="ExternalInput").ap()
    wv = nc.dram_tensor("wv", (D, D), bf16, kind="ExternalInput").ap()
    # valid01[k, q] = 1.0 where key k is valid for sorted query q, else 0.0
    maskT = nc.dram_tensor("maskT", (S, S), bf16, kind="ExternalInput").ap()
    out = nc.dram_tensor("out", (S, D), bf16, kind="ExternalOutput").ap()

    with ExitStack() as ctx:
        tc = ctx.enter_context(tile.TileContext(nc))
        const = ctx.enter_context(tc.tile_pool(name="const", bufs=1))
        persist = ctx.enter_context(tc.tile_pool(name="persist", bufs=1))
        wpool = ctx.enter_context(tc.tile_pool(name="wpool", bufs=1))
        ppool = ctx.enter_context(tc.tile_pool(name="ppool", bufs=1, space="PSUM"))
        # 4 buffers per kc-specific e tag: two head-pairs in flight, so
        # exp of pair p never waits on AV reads of pair p-1 (slot WAR).
        epool = ctx.enter_context(tc.tile_pool(name="epool", bufs=2))
        mpool = ctx.enter_context(tc.tile_pool(name="mpool", bufs=2))

        NB = 512  # max psum-bank columns (fp32) per matmul

        def mm(out_ap, lhsT, rhs, base, start, stop):
            # split a wide matmul into <=512-col pieces so each PE write
            # stays inside one PSUM bank. base = column offset of out_ap[0]
            # within its tile (bank alignment reference).
            w = rhs.shape[-1]
            off = 0
            while off < w:
                step = min(NB - ((base + off) % NB), w - off)
                nc.tensor.matmul(
                    out_ap[:, off : off + step],
                    lhsT,
                    rhs[:, off : off + step],
                    start=start,
                    stop=stop,
                )
                off += step

        ident = const.tile([P, P], fp32)
        make_identity(nc, ident[:])
        ident_bf = const.tile([P, P], bf16)
        nc.vector.tensor_copy(ident_bf[:], ident[:])

        rep_cm = tc.For_i(0, reps, 1) if reps > 1 else None
        if rep_cm is not None:
            ctx.enter_context(rep_cm)

        qt_sb = [persist.tile([P, S], bf16, tag=f"qt{i}", name=f"qt{i}") for i in range(DC)]
        kt_sb = [persist.tile([P, S], bf16, tag=f"kt{i}", name=f"kt{i}") for i in range(DC)]
        va_sb = [persist.tile([P, H * (DH + 1)], bf16, tag=f"va{i}", name=f"va{i}") for i in range(KC)]
        # mask tiles hold only the predicated [c0, cv) column range
        mk_sb = [
            persist.tile([P, max(pred_end[i] - col_start[i], 32)], bf16,
                         tag=f"mk{i}", name=f"mk{i}")
            for i in range(KC)
        ]

        # ---- input loads. x and W fully resident (bf16, 2MB each) ----
        def load_x(x_dram, pfx):
            return [wpool.tile([P, S], bf16, tag=f"x{pfx}{i}", name=f"x{pfx}{i}") for i in range(DC)]

        def load_w(w_dram, pfx):
            return [wpool.tile([P, D], bf16, tag=f"w{pfx}{i}", name=f"w{pfx}{i}") for i in range(DC)]

        xq_sb, wq_sb = load_x(xqT, "q"), load_w(wq, "q")
        xk_sb, wk_sb = load_x(xkT, "k"), load_w(wk, "k")
        xv_sb, wv_sb = load_x(xvT, "v"), load_w(wv, "v")
        # V loads first: the V projection phase runs first so that AV
        # matmuls can consume e-tiles promptly in the software pipeline.
        # One serial queue at HBM rate delivers V by ~12us (vs 27.5us of
        # V-projection PE work), Q by ~24us, K by ~36us -- just in time
        # for the pipeline's consumption order.
        for dc in (range(DC) if "noload" not in ABLATE else []):
            nc.sync.dma_start(xv_sb[dc][:], xvT[dc * P : (dc + 1) * P, :])
            nc.sync.dma_start(wv_sb[dc][:], wv[dc * P : (dc + 1) * P, :])
        for dc in (range(DC) if "noload" not in ABLATE else []):
            nc.sync.dma_start(xq_sb[dc][:], xqT[dc * P : (dc + 1) * P, :])
            nc.sync.dma_start(wq_sb[dc][:], wq[dc * P : (dc + 1) * P, :])
        for kc in (range(KC) if "noload" not in ABLATE else []):
            c0, cv = col_start[kc], pred_end[kc]
            if cv > c0:
                # small strided loads; needed by the pulled kc0 score
                # group at the end of iteration 0 (~38us)
                nc.sync.dma_start(mk_sb[kc][:, 0 : cv - c0],
                                  maskT[kc * P : (kc + 1) * P, c0:cv])
        for dc in (range(DC) if "noload" not in ABLATE else []):
            nc.sync.dma_start(xk_sb[dc][:], xkT[dc * P : (dc + 1) * P, :])
            nc.sync.dma_start(wk_sb[dc][:], wk[dc * P : (dc + 1) * P, :])
        for kc in range(KC):
            va3 = va_sb[kc].rearrange("p (h d) -> p h d", d=DH + 1)
            nc.vector.memset(va3[:, :, DH], 1.0)

        # ---- software pipeline ----
        # Engines execute their instruction queues in EMISSION order, so
        # overlap must be established by interleaved emission:
        #   phase V:  V projection (8 kc chunks), evac on ACT
        #   iter 0:   Q/K projection chunk 0
        #   iter oc (1..7): AV second half of pair oc-2, finalize pair oc-2,
        #       then Q/K projection chunk oc WOVEN with scores for pair oc-1
        #       (exp on ACT + mask on Pool run concurrently with the proj
        #       matmuls), then AV first half of pair oc-1.
        #   tail: finish pairs 6 and 7.
        # Per iteration PE ~13us > ACT exp ~11.7us: exp fully hidden.
        pt = [0]

        def acc_tile():
            t = ppool.tile([P, S], fp32, tag="aA" if pt[0] % 2 == 0 else "aB",
                           name="acc")
            pt[0] += 1
            return t

        kcs = [kc for kc in range(KC) if col_start[kc] < S]
        nkc = len(kcs)
        outv = out.rearrange("(s p) d -> p s d", p=P)  # [128, KC, D]

        # per-kc e tiles sized to the live column range [c0, S); one tile
        # holds BOTH heads of the pair ([0,w) = even head, [w,2w) = odd)
        # so a single exp/mask instruction can cover the pair via a
        # stride-w 3D access pattern. bufs=2 keeps two pairs in flight.
        def e_tile(kc):
            w = S - col_start[kc]
            return epool.tile([P, 2 * w], bf16, tag=f"e{kc}", name=f"e{kc}")

        asb_tiles = {}  # p -> (asb_e, asb_o)

        def fin_a(p, att_e, att_o):
            # evacuate the attention accumulators to SBUF (frees the aA/aB
            # PSUM banks for this iteration's Q/K projection accumulators).
            # att rows 0:64 = O^T unnormalized, row 64 = Z; bf16 throughout.
            asb_e = mpool.tile([DH + 1, S], bf16, tag="asbE")
            asb_o = mpool.tile([DH + 1, S], bf16, tag="asbO")
            nc.vector.tensor_copy(asb_e[:], att_e[:])
            nc.vector.tensor_copy(asb_o[:], att_o[:])
            asb_tiles[p] = (asb_e, asb_o)

        def fin_b(p):
            # transpose + normalize + store both heads of pair p. Emitted
            # mid-iteration (after the Q evac) so the trs allocation below
            # never blocks the PE at an iteration boundary. One [P, 2KC*128]
            # bf16 staging tile holds both heads (2 PSUM banks, 8 transpose
            # stripes per bank; each [128, 65] output stays inside a bank).
            asb_pair = asb_tiles.pop(p)
            trs = ppool.tile([P, 2 * KC * TRS], bf16, tag="aA", name="trs")
            tr3 = trs.rearrange("p (s d) -> p s d", d=TRS)
            for hh in range(2):
                asb = asb_pair[hh]
                for s in range(KC):
                    nc.tensor.transpose(
                        tr3[:, hh * KC + s, 0 : DH + 1],
                        asb[:, s * P : (s + 1) * P],
                        ident_bf[: DH + 1, : DH + 1],
                    )
            rz = mpool.tile([P, 2 * KC], fp32, tag="rz")
            nc.vector.reciprocal(rz[:], tr3[:, :, DH])
            ot = mpool.tile([P, 2 * KC * DH], bf16, tag="ot")
            ot3 = ot.rearrange("p (s d) -> p s d", d=DH)
            # single broadcast multiply instead of 16 per-chunk scalar muls
            rzb = rz[:].unsqueeze(2).broadcast_to((P, 2 * KC, DH))
            nc.vector.tensor_mul(ot3[:, :, :], tr3[:, :, 0:DH], rzb)
            if "nodma" not in ABLATE:
                # SP's HWDGE queue is idle once the input loads finish
                h = 2 * p
                nc.sync.dma_start(outv[:, :, h * DH : (h + 1) * DH],
                                  ot3[:, 0:KC, :])
                nc.sync.dma_start(outv[:, :, (h + 1) * DH : (h + 2) * DH],
                                  ot3[:, KC : 2 * KC, :])

        # ---- phase V: V projection (out[k, d] per kc chunk) ----
        # V accumulators live on the s0/s1 score-ring slots (identical
        # 2-bank shape) instead of the aA/aB accumulator ring: the score
        # ring frees EARLY in the previous repetition of the timing loop
        # (after the last pair's exp), so the next repetition's V matmuls
        # overlap the previous repetition's ACT-bound tail and drain
        # instead of waiting for its final finalize readers on aA/aB.
        if "noproj" not in ABLATE:
            for kc in range(KC):
                acc = ppool.tile([P, 2 * NB], fp32, tag=f"s{kc % 2}",
                                 name="vacc")
                for dc in range(DC):
                    if "nomm" in ABLATE:
                        break
                    mm(acc[:], xv_sb[dc][:, kc * P : (kc + 1) * P], wv_sb[dc][:],
                       0, dc == 0, dc == DC - 1)
                if "nomm" not in ABLATE:
                    dst = va_sb[kc].rearrange("p (h d) -> p h d", d=DH + 1)[:, :, 0:DH]
                    nc.scalar.copy(dst, acc[:].rearrange("p (h d) -> p h d", d=DH))

        # per-pair attention state carried across iterations
        att_tiles = {}  # p -> (att_e, att_o)

        # scores use a 2-deep ring of two-bank [P, 2*512] fp32 PSUM tiles
        # (tags s0/s1), each holding one <=512-col piece of BOTH heads
        # (even head at columns [0,512), odd at [512,1024)): one exp
        # instruction drains the whole pair piece.
        sct = [0]

        def sc_tile():
            t = ppool.tile([P, 2 * NB], fp32, tag=f"s{sct[0] % 2}", name="sc")
            sct[0] += 1
            return t

        def emit_scores(p, kc, i):
            # scores (piece ring) + exp (ACT) + mask (Pool) for pair p,
            # chunk kc. K=64 row-tiled pairs: the e/o matmuls sit at PE row
            # offsets 0/64 and execute concurrently on the array.
            c0 = col_start[kc]
            cv = pred_end[kc]
            w = S - c0
            n = -(-w // NB)
            bounds = [c0 + (w * j // n) // 32 * 32 for j in range(n)] + [S]
            w = S - c0
            e_pair = e_tile(kc)
            e2 = e_pair.rearrange("p (h w) -> p h w", h=2)
            # adjacent head matmuls target disjoint PE row-tiles (0:64 /
            # 64:128) so the hardware packs them into the array
            # concurrently; both write one 2-bank PSUM tile, drained by a
            # SINGLE exp (and masked by a single Pool mul) per piece.
            for j in range(n):
                lo, hi = bounds[j], bounds[j + 1]
                pw = hi - lo
                sc = sc_tile()
                sc2 = sc.rearrange("p (h w) -> p h w", h=2)
                for hh, lo_part in ((0, 0), (1, DH)):
                    nc.tensor.matmul(
                        sc2[:, hh, 0:pw],
                        kt_sb[p][lo_part : lo_part + DH, kc * P : (kc + 1) * P],
                        qt_sb[p][lo_part : lo_part + DH, lo:hi],
                        start=True, stop=True,
                    )
                nc.scalar.activation(e2[:, :, lo - c0 : hi - c0], sc2[:, :, 0:pw],
                                     AF.Exp, scale=0.125)
                mlo, mhi = max(lo, c0), min(hi, cv)
                if mlo < mhi and "nopred" not in ABLATE:
                    mkb = mk_sb[kc][:, mlo - c0 : mhi - c0].unsqueeze(1)\
                        .broadcast_to((P, 2, mhi - mlo))
                    nc.gpsimd.tensor_mul(
                        e2[:, :, mlo - c0 : mhi - c0],
                        e2[:, :, mlo - c0 : mhi - c0],
                        mkb,
                    )
            return e_pair, e2

        def emit_av(p, kc, i, e_pair, e2):
            att_e, att_o = att_tiles[p]
            he, ho = 2 * p, 2 * p + 1
            c0 = col_start[kc]
            w = S - c0
            mm(att_e[:, c0:], va_sb[kc][:, he * (DH + 1) : (he + 1) * (DH + 1)],
               e_pair[:, 0:w], c0, i == 0, i == nkc - 1)
            mm(att_o[:, c0:], va_sb[kc][:, ho * (DH + 1) : (ho + 1) * (DH + 1)],
               e_pair[:, w : 2 * w], c0, i == 0, i == nkc - 1)

        def proj_chunk_pieces(oc, w_sb, xf, dst_sb, evac_engine):
            """Per-dc matmul emitters for projection output chunk oc, plus
            the evacuation emitter."""
            acc = acc_tile()
            def mk(dc):
                def f():
                    if "nomm" not in ABLATE:
                        mm(acc[:], w_sb[dc][:, oc * P : (oc + 1) * P], xf[dc][:],
                           0, dc == 0, dc == DC - 1)
                return f
            pieces = [mk(dc) for dc in range(DC)]
            def evac():
                if "nomm" in ABLATE:
                    return
                if evac_engine == "pool":
                    nc.gpsimd.tensor_copy(dst_sb[oc][:], acc[:])
                elif evac_engine == "act":
                    nc.scalar.copy(dst_sb[oc][:], acc[:])
                else:
                    nc.vector.tensor_copy(dst_sb[oc][:], acc[:])
            return pieces, evac

        NITER = DC  # 8 head pairs / proj chunks
        # AV schedule: group kc0 of pair p is score+exp'd at the END of
        # iteration p (right after its qt/kt evacs), so the ACT engine has
        # work across the iteration boundary; groups kc1..7 weave through
        # iteration p+1; AVs kc0..2 weave at j>=5, kc3..5 follow the weave,
        # kc6..7 (narrowest) lead iteration p+2.
        N_TAIL = 2
        N_PULL = 2  # score groups of pair p emitted at the end of iter p
        ehold = {}   # p -> list of (kc, i, e_e, e_o) awaiting AV next iter
        pulled = {}  # p -> [(kc, i, e_e, e_o)] score groups emitted early

        do_attn = "noattn" not in ABLATE
        do_fin = "notr" not in ABLATE

        def alloc_att(p):
            att_tiles[p] = (
                ppool.tile([DH + 1, S], fp32, tag="aA", name="att_e"),
                ppool.tile([DH + 1, S], fp32, tag="aB", name="att_o"),
            )

        for it in range(NITER + 2):
            # -- tail: AV of pair it-2's narrow kcs, then evac its accs --
            p2 = it - 2
            if do_attn and 0 <= p2 < DC:
                for (kc, i, e_e, e_o) in ehold.pop(p2):
                    emit_av(p2, kc, i, e_e, e_o)
                if do_fin:
                    att_e, att_o = att_tiles.pop(p2)
                    fin_a(p2, att_e, att_o)
            if it > NITER:
                if do_attn and do_fin and NITER - 1 >= 0:
                    fin_b(NITER - 1)  # last pair finalizes in the drain iter
                continue
            p = it - 1
            if it < NITER and "noproj" not in ABLATE:
                qp, qevac = proj_chunk_pieces(it, wq_sb, xq_sb, qt_sb, "dve")
                kp, kevac = proj_chunk_pieces(it, wk_sb, xk_sb, kt_sb, "dve")
                pieces = qp + kp
                evacs = {len(qp) - 1: qevac, len(qp) + len(kp) - 1: kevac}
            else:
                pieces, evacs = [], {}
            have_att = do_attn and 0 <= p < DC
            es = pulled.pop(p, [])
            sc_groups = list(enumerate(kcs))[len(es):] if have_att else []
            npz = len(pieces)
            gq = list(sc_groups)
            for j in range(max(8, (npz + 1) // 2)):
                if gq:
                    i, kc = gq.pop(0)
                    es.append((kc, i) + emit_scores(p, kc, i))
                for pi in (2 * j, 2 * j + 1):
                    if pi < npz:
                        pieces[pi]()
                        if pi in evacs:
                            evacs[pi]()
                if do_attn and do_fin and j == 3 and 0 <= p2 < DC:
                    fin_b(p2)
                if have_att and j >= 5 and j - 5 < len(es):
                    if j == 5:
                        alloc_att(p)
                    emit_av(p, *es[j - 5])
            # -- post-weave AVs (kc3..kc5) --
            if have_att and es:
                if p not in att_tiles:
                    alloc_att(p)
                done = max(0, min(3, len(es) - N_TAIL))
                for (kc, i, e_e, e_o) in es[done : max(done, len(es) - N_TAIL)]:
                    emit_av(p, kc, i, e_e, e_o)
                ehold[p] = es[max(done, len(es) - N_TAIL):]
            # -- pull the widest score groups of pair `it` (qt/kt just
            # evac'd) so ACT has exp work across the iteration boundary --
            if do_attn and 0 <= it < DC and kcs:
                pulled[it] = [
                    (kc, i) + emit_scores(it, kc, i)
                    for i, kc in list(enumerate(kcs))[:N_PULL]
                ]

    nc.compile()
    return nc


_NC_CACHE = {}
_RUNNER_CACHE = {}
_PREP_JIT = []
_LAST_IN_MAPS = None


def _get_nc(col_start, pred_end):
    key = (tuple(col_start), tuple(pred_end))
    if key not in _NC_CACHE:
        _NC_CACHE[key] = _build_nc(list(col_start), list(pred_end))
    return _NC_CACHE[key]


def _get_runner(nc):
    """Build the sharded PJRT callable ONCE per nc and reuse it across
    kernel() calls -- run_bass_kernel_spmd re-traces and re-jits on every
    invocation, which costs seconds of host time per call."""
    if nc in _RUNNER_CACHE:
        return _RUNNER_CACHE[nc]
    import jax
    import concourse.mybir as mybir
    from jax.sharding import Mesh, PartitionSpec
    from jax.experimental.shard_map import shard_map
    from concourse import bass2jax

    bass2jax.install_neuronx_cc_hook()
    partition_name = nc.partition_id_tensor.name if nc.partition_id_tensor else None
    in_names, out_names, out_avals = [], [], []
    for alloc in nc.m.functions[0].allocations:
        if not isinstance(alloc, mybir.MemoryLocationSet):
            continue
        if not alloc.memorylocations:
            continue
        name = alloc.memorylocations[0].name
        if alloc.kind == "ExternalInput":
            if name != partition_name:
                in_names.append(name)
        elif alloc.kind == "ExternalOutput":
            out_names.append(name)
            shape = tuple(alloc.tensor_shape)
            dtype = mybir.dt.np(alloc.dtype)
            out_avals.append(jax.core.ShapedArray(shape, dtype))
    n_params = len(in_names)
    all_in = in_names + out_names + ([partition_name] if partition_name else [])

    def _body(*args):
        operands = list(args)
        if partition_name is not None:
            operands.append(bass2jax.partition_id_tensor())
        outs = bass2jax._bass_exec_p.bind(
            *operands,
            out_avals=tuple(out_avals),
            in_names=tuple(all_in),
            out_names=tuple(out_names),
            lowering_input_output_aliases=(),
            sim_require_finite=True,
            sim_require_nnan=True,
            nc=nc,
        )
        return tuple(outs)

    devices = jax.devices()[:N_CORES]
    mesh = Mesh(np.asarray(devices), ("core",))
    n_outs = len(out_names)
    sharded = jax.jit(
        shard_map(
            _body,
            mesh=mesh,
            in_specs=(PartitionSpec("core"),) * (n_params + n_outs),
            out_specs=(PartitionSpec("core"),) * n_outs,
            check_rep=False,
        ),
        keep_unused=True,
    )
    # every element of every output is written by the kernel, so the
    # pre-zeroed output operands can be device-resident and reused.
    zeros = [
        jax.device_put(
            np.zeros((N_CORES * a.shape[0], *a.shape[1:]), a.dtype)
        )
        for a in out_avals
    ]
    for z in zeros:
        z.block_until_ready()

    def run(in_maps):
        concat_in = [
            np.concatenate([np.asarray(m[name]) for m in in_maps], axis=0)
            for name in in_names
        ]
        out_arrs = sharded(*concat_in, *zeros)
        return [
            {
                name: np.asarray(out_arrs[i]).reshape(
                    N_CORES, *out_avals[i].shape
                )[c]
                for i, name in enumerate(out_names)
            }
            for c in range(N_CORES)
        ]

    _RUNNER_CACHE[nc] = run
    return run


def _get_prep_jit():
    """jax-CPU jitted input prep (transpose + bf16 cast + mask build) --
    multi-threaded XLA beats the serial numpy path by ~5x."""
    if not _PREP_JIT:
        import jax
        import jax.numpy as jnp

        cpu = jax.devices("cpu")[0]

        def f(q, k, v, orders, vs):
            qs = jnp.take_along_axis(q, orders[:, :, None], axis=1)
            xqT = jnp.swapaxes(qs, 1, 2).astype(jnp.bfloat16)
            xkT = jnp.swapaxes(k, 1, 2).astype(jnp.bfloat16)
            xvT = jnp.swapaxes(v, 1, 2).astype(jnp.bfloat16)
            kidx = jnp.arange(S, dtype=jnp.int32)
            maskT = (kidx[None, :, None] < vs[:, None, :]).astype(jnp.bfloat16)
            return xqT, xkT, xvT, maskT

        def fo(o_bf, inv):
            # bf16 device output -> fp32, unsorted back to query order
            return jnp.take_along_axis(
                o_bf.astype(jnp.float32), inv[:, :, None], axis=1
            )

        fj, foj = jax.jit(f), jax.jit(fo)

        def fin(*a):
            with jax.default_device(cpu):
                return fj(*a)

        def fout(*a):
            with jax.default_device(cpu):
                return foj(*a)

        _PREP_JIT.append(fin)
        _PREP_JIT.append(fout)
    return _PREP_JIT


def _prep(query, key, value, valid_len, Wq, Wk, Wv):
    import ml_dtypes

    bf = ml_dtypes.bfloat16
    orders = []
    vss = []
    col_start = [S] * KC
    pred_end = [0] * KC
    wqb, wkb, wvb = Wq.astype(bf), Wk.astype(bf), Wv.astype(bf)
    for b in range(B):
        vl = valid_len[b]
        vl2 = np.where(vl == 0, 1, vl).astype(np.int32)
        order = np.argsort(vl2, kind="stable")
        orders.append(order)
        vs = vl2[order]
        vss.append(vs)
        for kc in range(KC):
            need = vs > (kc * P)
            c0 = S if not need.any() else (int(np.argmax(need)) // 32) * 32
            col_start[kc] = min(col_start[kc], c0)
            full = vs >= ((kc + 1) * P)
            cv = S if not full.any() else int(np.argmax(full))
            pred_end[kc] = max(pred_end[kc], min(S, -(-cv // 32) * 32))
    fin, _ = _get_prep_jit()
    xqT, xkT, xvT, maskT = (
        np.asarray(a, dtype=bf)
        for a in fin(
            query, key, value,
            np.stack(orders).astype(np.int32),
            np.stack(vss).astype(np.int32),
        )
    )
    in_maps = [
        {
            "xqT": xqT[b],
            "xkT": xkT[b],
            "xvT": xvT[b],
            "wq": wqb,
            "wk": wkb,
            "wv": wvb,
            "maskT": maskT[b],
        }
        for b in range(B)
    ]
    return in_maps, orders, col_start, pred_end


def kernel(query, key, value, valid_len, Wq, Wk, Wv):
    query = np.asarray(query, dtype=np.float32)
    key = np.asarray(key, dtype=np.float32)
    value = np.asarray(value, dtype=np.float32)
    valid_len = np.asarray(valid_len, dtype=np.int32)
    Wq = np.asarray(Wq, dtype=np.float32)
    Wk = np.asarray(Wk, dtype=np.float32)
    Wv = np.asarray(Wv, dtype=np.float32)

    in_maps, orders, col_start, pred_end = _prep(
        query, key, value, valid_len, Wq, Wk, Wv
    )
    nc = _get_nc(col_start, pred_end)
    global _LAST_IN_MAPS
    _LAST_IN_MAPS = in_maps
    results = _get_runner(nc)(in_maps)

    _, fout = _get_prep_jit()
    o_stack = np.stack([results[b]["out"] for b in range(B)])
    invs = np.empty((B, S), dtype=np.int32)
    for b in range(B):
        invs[b][orders[b]] = np.arange(S, dtype=np.int32)
    outs = np.array(fout(o_stack, invs), dtype=np.float32)
    for b in range(B):
        zrows = np.where(valid_len[b] == 0)[0]
        if len(zrows):
            outs[b][zrows] = value[b].mean(axis=0) @ Wv
    return outs

